# revision 1
# baseline (speedup 1.0000x reference)
"""FastWorkingMemory (DeltaNet-style recurrence with vector learning rate) on 8 TRN2 cores.

Reference computation (B=4, T=2048, D=1024, H=8, d=128):
    q = x @ Wq.T ; k = l2norm(x @ Wk.T) ; v = l2norm(x @ Wv.T)   (per-head d=128)
    lr = sigmoid(x @ Wlr.T + b_lr)
    scan over t:  v_old = S k_t ; S += (lr_t * (v_t - v_old)) k_t^T ; o_t = S q_t
    y = o @ Wo.T

Sharding: core c -> batch b = c//2, heads hg = c%2 (4 heads each). Each core computes a
partial y (its heads' contribution through Wo); host sums the two partials per batch.

Device algorithm: chunked delta rule, chunk C=128. Per (head, chunk):
    A = K K^T strict-lower, G = K Q^T masked s<=t  ([s,t] layouts)
    Vold = K @ P            (P = S^T state, [j,i])
    R = lr * (V - Vold)
    U = (I + D)^-1 R,  D(X) = lr o (A_strict X)  -- truncated Neumann/Horner:
        Z'_k = -lr o (A (R + Z'_{k-1})),  U = R + Z'_M
        (keys are l2-normalized and incoherent; M=14 leaves ~1e-4 worst-case
         truncation, below the fp16/fp32r arithmetic floor)
    O^T = P^T Q^T + U^T G   (one PSUM accumulation group)
    P  += K_rows^T U
    y_chunk = O @ Wo_cols   (fp32r out-projection)

dtypes: projections/out-proj fp32r (tf32-class, 1 cyc/row at N>=256);
state matmuls fp32; Neumann matmuls fp16.
"""

import numpy as np

B, T, D, H = 4, 2048, 1024, 8
d = D // H
HPC = 4            # heads per core
DH = HPC * d       # 512: packed head width
C = 128            # scan chunk
W = 256            # projection window (t)
NWIN = T // W      # 8
NSUB = W // C      # 2 chunks per window
NJ = D // 128      # 8 contraction tiles
NEUMANN_ITERS = 12
EPS = 1e-12

_prog_cache = {}


def _build_program(debug=False):
    def ssl_h(h):
        return slice(h * 128, (h + 1) * 128)

    import concourse.mybir as mybir
    import concourse.tile as tile
    from concourse import bacc
    from concourse.masks import make_identity, make_upper_triangular

    f32 = mybir.dt.float32
    f32r = mybir.dt.float32r
    f16 = mybir.dt.float16
    Alu = mybir.AluOpType
    Act = mybir.ActivationFunctionType

    nc = bacc.Bacc("TRN2", target_bir_lowering=False, debug=False, num_devices=8)

    xT = nc.dram_tensor("xT", [D, T], f32r, kind="ExternalInput").ap()
    WqT = nc.dram_tensor("WqT", [D, DH], f32r, kind="ExternalInput").ap()
    WkT = nc.dram_tensor("WkT", [D, DH], f32r, kind="ExternalInput").ap()
    WvT = nc.dram_tensor("WvT", [D, DH], f32r, kind="ExternalInput").ap()
    WlT = nc.dram_tensor("WlT", [D, DH], f32r, kind="ExternalInput").ap()
    blr = nc.dram_tensor("blr", [1, DH], f32, kind="ExternalInput").ap()
    WoT = nc.dram_tensor("WoT", [DH, D], f32r, kind="ExternalInput").ap()
    y = nc.dram_tensor("y", [T, D], f32, kind="ExternalOutput").ap()
    dbg = {}
    if debug:
        for nm in ("kr0", "vr0", "lr0", "A0", "G0", "R0", "U0", "Ot0", "P0", "kt0", "qt0"):
            dbg[nm] = nc.dram_tensor("dbg_" + nm, [128, DH], f32, kind="ExternalOutput").ap()

    with tile.TileContext(nc) as tc:
        with (
            tc.tile_pool(name="consts", bufs=1) as consts,
            tc.tile_pool(name="weights", bufs=1) as wpool,
            tc.tile_pool(name="state", bufs=1) as state,
            tc.tile_pool(name="xwin", bufs=2) as xwin,
            tc.tile_pool(name="rows", bufs=2) as rows,
            tc.tile_pool(name="twin", bufs=2) as twin,
            tc.tile_pool(name="chunk", bufs=2) as chk,
            tc.tile_pool(name="nscratch", bufs=2) as nsc,
            tc.tile_pool(name="ps_work", bufs=4, space="PSUM") as ps_work,
            tc.tile_pool(name="ps_neu", bufs=2, space="PSUM") as ps_neu,
            tc.tile_pool(name="ps_proj", bufs=2, space="PSUM") as ps_proj,
        ):
            # ---- constants ----
            ident = consts.tile([128, 128], f32, tag="ident")
            make_identity(nc, ident)
            ident16 = consts.tile([128, 128], f16, tag="ident16")
            nc.gpsimd.tensor_copy(ident16[:], ident[:])
            ident32r = consts.tile([128, 128], f32r, tag="ident32r")
            nc.gpsimd.tensor_copy(ident32r[:], ident[:])
            maskA1 = consts.tile([128, 128], f32, tag="maskA1")  # 1 where s<t
            make_upper_triangular(nc, maskA1, val=1.0, diag=False)
            maskG1 = consts.tile([128, 128], f32, tag="maskG1")  # 1 where s<=t
            make_upper_triangular(nc, maskG1, val=1.0, diag=True)
            maskA = consts.tile([128, DH], f32, tag="maskA")
            maskG = consts.tile([128, DH], f32, tag="maskG")
            for h in range(HPC):
                nc.gpsimd.tensor_copy(maskA[:, h * 128:(h + 1) * 128], maskA1[:, :])
                nc.gpsimd.tensor_copy(maskG[:, h * 128:(h + 1) * 128], maskG1[:, :])
            ones_row = consts.tile([1, 128], f16, tag="ones_row")
            nc.vector.memset(ones_row[:], 1.0)
            blr_f32 = consts.tile([1, DH], f32, tag="blr_f32")
            nc.sync.dma_start(blr_f32[:], blr[:])
            blr_sb = consts.tile([1, DH], f16, tag="blr_sb")
            nc.gpsimd.tensor_copy(blr_sb[:], blr_f32[:])

            # ---- resident weights ----
            wq = [wpool.tile([128, DH], f32r, tag=f"wq{j}", name=f"wq{j}") for j in range(NJ)]
            wk = [wpool.tile([128, DH], f32r, tag=f"wk{j}", name=f"wk{j}") for j in range(NJ)]
            wv = [wpool.tile([128, DH], f32r, tag=f"wv{j}", name=f"wv{j}") for j in range(NJ)]
            wl = [wpool.tile([128, DH], f32r, tag=f"wl{j}", name=f"wl{j}") for j in range(NJ)]
            for j in range(NJ):
                eng_w = nc.scalar if j % 2 == 0 else nc.sync
                eng_w.dma_start(wk[j][:], WkT[j * 128:(j + 1) * 128, :])
                nc.gpsimd.dma_start(wv[j][:], WvT[j * 128:(j + 1) * 128, :])
                nc.gpsimd.dma_start(wl[j][:], WlT[j * 128:(j + 1) * 128, :])
                nc.gpsimd.dma_start(wq[j][:], WqT[j * 128:(j + 1) * 128, :])
            wo = [wpool.tile([128, D], f32r, tag=f"wo{h}", name=f"wo{h}") for h in range(HPC)]
            for h in range(HPC):
                nc.gpsimd.dma_start(wo[h][:], WoT[h * 128:(h + 1) * 128, :])

            # ---- state ----
            # P = S^T per head; two independent head-group streams (2 heads each)
            P2 = [state.tile([128, 256], f32, tag=f"P2_{u}", name=f"P2_{u}") for u in range(2)]
            P2h = [state.tile([128, 256], f16, tag=f"P2h_{u}", name=f"P2h_{u}") for u in range(2)]
            for u in range(2):
                nc.vector.memset(P2[u][:], 0.0)
                nc.vector.memset(P2h[u][:], 0.0)

            def norm_part1(ps, raw, ss, col):
                """drain psum, square, reduce sumsq per head into ss[:, col:col+HPC]."""
                nc.scalar.copy(raw[:], ps[:])
                sq = nsc.tile([128, DH], f32, tag="nsq")
                nc.gpsimd.tensor_tensor(sq[:], raw[:], raw[:], Alu.mult)
                nc.vector.tensor_reduce(
                    ss[:, col:col + HPC], sq[:].rearrange("p (h i) -> p h i", h=HPC),
                    axis=mybir.AxisListType.X, op=Alu.add)

            def norm_part2(ss, rcp, n):
                """rcp[:, :n] = 1 / max(sqrt(ss[:, :n]), eps)."""
                nc.scalar.activation(rcp[:, :n], ss[:, :n], Act.Sqrt)
                nc.vector.tensor_scalar(
                    out=rcp[:, :n], in0=rcp[:, :n], scalar1=EPS, scalar2=None,
                    op0=Alu.max)
                nc.vector.reciprocal(rcp[:, :n], rcp[:, :n])

            def norm_scale(raw, rcp, col, out_rows):
                for h in range(HPC):
                    hsl = slice(h * 128, (h + 1) * 128)
                    nc.gpsimd.tensor_tensor(
                        out_rows[:, hsl], raw[:, hsl],
                        rcp[:, col + h:col + h + 1].to_broadcast((128, 128)),
                        Alu.mult)

            def emit_proj(w):
                # ---- load x^T window ----
                xt = [xwin.tile([128, W], f32r, tag=f"xt{j}", name=f"xt{j}_{w}") for j in range(NJ)]
                for j in range(NJ):
                    eng_x = nc.sync if j % 2 == 0 else nc.scalar
                    eng_x.dma_start(
                        xt[j][:], xT[j * 128:(j + 1) * 128, w * W:(w + 1) * W])

                kr = [rows.tile([128, DH], f16, tag=f"kr{s}", name=f"kr{s}_{w}") for s in range(NSUB)]
                vr = [rows.tile([128, DH], f32r, tag=f"vr{s}", name=f"vr{s}_{w}") for s in range(NSUB)]
                lr = [rows.tile([128, DH], f32, tag=f"lr{s}", name=f"lr{s}_{w}") for s in range(NSUB)]
                ln = [rows.tile([128, DH], f32, tag=f"ln{s}", name=f"ln{s}_{w}") for s in range(NSUB)]
                ktw = twin.tile([128, HPC * W], f16, tag="ktw")   # [j, (h, t_w)]
                qtw = twin.tile([128, HPC * W], f16, tag="qtw")
                kt3 = ktw[:].rearrange("p (h t) -> p h t", h=HPC)
                qt3 = qtw[:].rearrange("p (h t) -> p h t", h=HPC)

                def proj(wts, tsl, extra_bias=False):
                    ps = ps_proj.tile([128, DH], f32, tag="proj")
                    for j in range(NJ):
                        nc.tensor.matmul(
                            ps[:], xt[j][:, tsl], wts[j][:],
                            start=(j == 0),
                            stop=(j == NJ - 1 and not extra_bias))
                    if extra_bias:
                        nc.tensor.matmul(
                            ps[:], ones_row[:], blr_sb[:], start=False, stop=True)
                    return ps

                # phases: K (sqrt) -> Q+transposes (copies) -> V (sqrt) -> LR (sigmoid)
                # ACT table swaps stay at ~2/window; copies live in every set
                ssK = nsc.tile([128, 2 * HPC], f32, tag="ssK")
                rcpK = nsc.tile([128, 2 * HPC], f32, tag="rcpK")
                rawK = [nsc.tile([128, DH], f32, tag=f"rawK{s}", name=f"rawK{s}_{w}") for s in range(NSUB)]
                for s in range(NSUB):
                    norm_part1(proj(wk, slice(s * 128, (s + 1) * 128)), rawK[s], ssK, s * HPC)
                norm_part2(ssK, rcpK, NSUB * HPC)
                for s in range(NSUB):
                    norm_scale(rawK[s], rcpK, s * HPC, kr[s])

                for s in range(NSUB):
                    psq = proj(wq, slice(s * 128, (s + 1) * 128))
                    qr = nsc.tile([128, DH], f16, tag="qr")
                    nc.scalar.copy(qr[:], psq[:])
                    pst = ps_work.tile([128, DH], f16, tag="work")
                    for h in range(HPC):
                        hsl = slice(h * 128, (h + 1) * 128)
                        nc.tensor.transpose(pst[:, hsl], kr[s][:, hsl], ident16[:])
                    nc.scalar.copy(
                        kt3[:, :, s * 128:(s + 1) * 128],
                        pst[:].rearrange("p (h t) -> p h t", h=HPC))
                    pst2 = ps_work.tile([128, DH], f16, tag="work")
                    for h in range(HPC):
                        hsl = slice(h * 128, (h + 1) * 128)
                        nc.tensor.transpose(pst2[:, hsl], qr[:, hsl], ident16[:])
                    nc.scalar.copy(
                        qt3[:, :, s * 128:(s + 1) * 128],
                        pst2[:].rearrange("p (h t) -> p h t", h=HPC))

                ssV = nsc.tile([128, 2 * HPC], f32, tag="ssV")
                rcpV = nsc.tile([128, 2 * HPC], f32, tag="rcpV")
                rawV = [nsc.tile([128, DH], f32, tag=f"rawV{s}", name=f"rawV{s}_{w}") for s in range(NSUB)]
                for s in range(NSUB):
                    norm_part1(proj(wv, slice(s * 128, (s + 1) * 128)), rawV[s], ssV, s * HPC)
                norm_part2(ssV, rcpV, NSUB * HPC)
                nc.vector.tensor_scalar(
                    out=rcpV[:, :NSUB * HPC], in0=rcpV[:, :NSUB * HPC],
                    scalar1=-1.0, scalar2=None, op0=Alu.mult)
                for s in range(NSUB):
                    norm_scale(rawV[s], rcpV, s * HPC, vr[s])

                for s in range(NSUB):
                    psl = proj(wl, slice(s * 128, (s + 1) * 128), extra_bias=True)
                    nc.scalar.activation(lr[s][:], psl[:], Act.Sigmoid)
                    nc.gpsimd.tensor_scalar(
                        out=ln[s][:], in0=lr[s][:], scalar1=-1.0, scalar2=None,
                        op0=Alu.mult)

                if debug and w == 0:
                    nc.sync.dma_start(dbg["kr0"][:], kr[0][:])
                    nc.sync.dma_start(dbg["vr0"][:], vr[0][:])
                    nc.sync.dma_start(dbg["lr0"][:], lr[0][:])
                    kt_dump = nsc.tile([128, DH], f32, tag="ktdump")
                    qt_dump = nsc.tile([128, DH], f32, tag="qtdump")
                    for h in range(HPC):
                        nc.vector.tensor_copy(kt_dump[:, h * 128:(h + 1) * 128], kt3[:, h, 0:128])
                        nc.vector.tensor_copy(qt_dump[:, h * 128:(h + 1) * 128], qt3[:, h, 0:128])
                    nc.sync.dma_start(dbg["kt0"][:], kt_dump[:])
                    nc.sync.dma_start(dbg["qt0"][:], qt_dump[:])

                return kr, vr, lr, ln, kt3, qt3

            def emit_scan(w, tiles):
                kr, vr, lr, ln, kt3, qt3 = tiles
                # ---- scan chunks (two interleaved head-group streams) ----
                for s in range(NSUB):
                    csl = slice(s * 128, (s + 1) * 128)
                    STR = (slice(0, 256), slice(256, 512))
                    HH = ((0, 1), (2, 3))

                    A2, G2, R2, Rb2, zb2, U2, Ot2 = [], [], [], [], [], [], []
                    for u in range(2):
                        ssl = STR[u]
                        # A = K K^T strict-lower -> fp16
                        psA = ps_work.tile([128, 256], f32, tag="work", name=f"psA{u}_{w}_{s}")
                        for j, h in enumerate(HH[u]):
                            hsl = slice(j * 128, (j + 1) * 128)
                            nc.tensor.matmul(
                                psA[:, hsl], kt3[:, h, csl], kt3[:, h, csl],
                                start=True, stop=True)
                        A4 = chk.tile([128, 256], f16, tag=f"A4_{u}", name=f"A4_{u}_{w}_{s}")
                        nc.vector.tensor_tensor(A4[:], psA[:], maskA[:, ssl], Alu.mult)
                        A2.append(A4)

                        # G = K Q^T masked s<=t (f32)
                        psG = ps_work.tile([128, 256], f32, tag="work", name=f"psG{u}_{w}_{s}")
                        for j, h in enumerate(HH[u]):
                            hsl = slice(j * 128, (j + 1) * 128)
                            nc.tensor.matmul(
                                psG[:, hsl], kt3[:, h, csl], qt3[:, h, csl],
                                start=True, stop=True)
                        G4 = chk.tile([128, 256], f16, tag=f"G4_{u}", name=f"G4_{u}_{w}_{s}")
                        nc.vector.tensor_tensor(G4[:], psG[:], maskG[:, ssl], Alu.mult)
                        G2.append(G4)

                        # Vold = K @ P (rows), R = lr*(V - Vold)
                        psVo = ps_work.tile([128, 256], f32, tag="work", name=f"psVo{u}_{w}_{s}")
                        nc.tensor.matmul(
                            psVo[:], ident32r[:], vr[s][:, ssl],
                            start=True, stop=False)
                        for j, h in enumerate(HH[u]):
                            hsl = slice(j * 128, (j + 1) * 128)
                            nc.tensor.matmul(
                                psVo[:, hsl], kt3[:, h, csl], P2h[u][:, hsl],
                                start=False, stop=True)
                        R4 = chk.tile([128, 256], f32, tag=f"R4_{u}", name=f"R4_{u}_{w}_{s}")
                        nc.vector.tensor_tensor(R4[:], ln[s][:, ssl], psVo[:], Alu.mult)
                        Rb = chk.tile([128, 256], f16, tag=f"Rb_{u}", name=f"Rb_{u}_{w}_{s}")
                        nc.gpsimd.tensor_copy(Rb[:], R4[:])
                        R2.append(R4)
                        Rb2.append(Rb)
                        zb2.append(None)

                    # Neumann/Horner, streams interleaved per iteration:
                    # Z'_k = -lr o (A @ (R + Z'_{k-1}))
                    for it in range(NEUMANN_ITERS):
                        psN2 = []
                        for u in range(2):
                            psN = ps_neu.tile([128, 256], f32, tag="neu", name=f"psN{u}_{w}_{s}_{it}")
                            for j in range(2):
                                hsl = slice(j * 128, (j + 1) * 128)
                                nc.tensor.matmul(
                                    psN[:, hsl], A2[u][:, hsl], Rb2[u][:, hsl],
                                    start=True, stop=(zb2[u] is None))
                                if zb2[u] is not None:
                                    nc.tensor.matmul(
                                        psN[:, hsl], A2[u][:, hsl], zb2[u][:, hsl],
                                        start=False, stop=True)
                            psN2.append(psN)
                        for u in range(2):
                            zb_new = chk.tile([128, 256], f16, tag=f"zb_{u}", name=f"zb_{u}_{w}_{s}_{it}")
                            nc.vector.tensor_tensor(zb_new[:], ln[s][:, STR[u]], psN2[u][:], Alu.mult)
                            zb2[u] = zb_new

                    for u in range(2):
                        U4 = chk.tile([128, 256], f16, tag=f"U4_{u}", name=f"U4_{u}_{w}_{s}")
                        nc.gpsimd.tensor_tensor(U4[:], R2[u][:], zb2[u][:], Alu.add)
                        U2.append(U4)

                        # O^T = P^T Q^T + U^T G   [i, (h,t)]
                        psO = ps_work.tile([128, 256], f32, tag="work", name=f"psO{u}_{w}_{s}")
                        for j, h in enumerate(HH[u]):
                            hsl = slice(j * 128, (j + 1) * 128)
                            nc.tensor.matmul(
                                psO[:, hsl], P2h[u][:, hsl], qt3[:, h, csl],
                                start=True, stop=False)
                            nc.tensor.matmul(
                                psO[:, hsl], U4[:, hsl], G2[u][:, hsl],
                                start=False, stop=True)
                        Ot = chk.tile([128, 256], f32r, tag=f"Ot_{u}", name=f"Ot_{u}_{w}_{s}")
                        nc.scalar.copy(Ot[:], psO[:])
                        Ot2.append(Ot)

                        # P += K_rows^T U
                        psP = ps_work.tile([128, 256], f32, tag="work", name=f"psP{u}_{w}_{s}")
                        for j, h in enumerate(HH[u]):
                            hsl = slice(j * 128, (j + 1) * 128)
                            nc.tensor.matmul(
                                psP[:, hsl], kr[s][:, ssl_h(h)], U4[:, hsl],
                                start=True, stop=True)
                        nc.vector.tensor_tensor(P2[u][:], P2[u][:], psP[:], Alu.add)
                        nc.scalar.copy(P2h[u][:], P2[u][:])

                    # y_chunk = O @ Wo_cols   [t, o]
                    t0 = w * W + s * 128
                    for ot in range(2):
                        osl = slice(ot * 512, (ot + 1) * 512)
                        psy = ps_work.tile([128, 512], f32, tag="work", name=f"psy{ot}_{w}_{s}")
                        for h in range(HPC):
                            u, j = divmod(h, 2)
                            hsl = slice(j * 128, (j + 1) * 128)
                            nc.tensor.matmul(
                                psy[:], Ot2[u][:, hsl], wo[h][:, osl],
                                start=(h == 0), stop=(h == HPC - 1))
                        y_sb = chk.tile([128, 512], f32, tag=f"y_sb{ot}", name=f"ysb{ot}_{w}_{s}")
                        nc.scalar.copy(y_sb[:], psy[:])
                        nc.sync.dma_start(y[t0:t0 + 128, osl], y_sb[:])


            for w in range(NWIN):
                emit_scan(w, emit_proj(w))

    nc.compile()
    return nc


def get_program(debug=False):
    key = "nc_dbg" if debug else "nc"
    if key not in _prog_cache:
        _prog_cache[key] = _build_program(debug)
    return _prog_cache[key]


def kernel(x, Wq, Wk, Wv, Wo, Wlr, b_lr):
    from concourse import bass_utils

    nc = get_program()
    x = np.asarray(x, np.float32)
    Wq = np.asarray(Wq, np.float32)
    Wk = np.asarray(Wk, np.float32)
    Wv = np.asarray(Wv, np.float32)
    Wo = np.asarray(Wo, np.float32)
    Wlr = np.asarray(Wlr, np.float32)
    b_lr = np.asarray(b_lr, np.float32)

    in_maps = []
    for c in range(8):
        b, hg = divmod(c, 2)
        rs = slice(hg * DH, (hg + 1) * DH)   # head-sliced output rows of W*
        in_maps.append({
            "xT": np.ascontiguousarray(x[b].T),
            "WqT": np.ascontiguousarray(Wq[rs, :].T),
            "WkT": np.ascontiguousarray(Wk[rs, :].T),
            "WvT": np.ascontiguousarray(Wv[rs, :].T),
            "WlT": np.ascontiguousarray(Wlr[rs, :].T),
            "blr": np.ascontiguousarray(b_lr[rs][None, :]),
            "WoT": np.ascontiguousarray(Wo[:, rs].T),
        })
    res = bass_utils.run_bass_kernel_spmd(nc, in_maps, core_ids=list(range(8)))
    out = np.empty((B, T, D), np.float32)
    for b in range(B):
        out[b] = res.results[2 * b]["y"] + res.results[2 * b + 1]["y"]
    return out



# revision 10
# speedup vs baseline: 1.0812x; 1.0812x over previous
"""FastWorkingMemory (DeltaNet-style recurrence with vector learning rate) on 8 TRN2 cores.

Reference computation (B=4, T=2048, D=1024, H=8, d=128):
    q = x @ Wq.T ; k = l2norm(x @ Wk.T) ; v = l2norm(x @ Wv.T)   (per-head d=128)
    lr = sigmoid(x @ Wlr.T + b_lr)
    scan over t:  v_old = S k_t ; S += (lr_t * (v_t - v_old)) k_t^T ; o_t = S q_t
    y = o @ Wo.T

Sharding: core c -> batch b = c//2, heads hg = c%2 (4 heads each). Each core computes a
partial y (its heads' contribution through Wo); host sums the two partials per batch.

Device algorithm: chunked delta rule, chunk C=128. Per (head, chunk):
    A = K K^T strict-lower, G = K Q^T masked s<=t  ([s,t] layouts)
    Vold = K @ P            (P = S^T state, [j,i])
    R = lr * (V - Vold)
    U = (I + D)^-1 R,  D(X) = lr o (A_strict X)  -- truncated Neumann/Horner:
        Z'_k = -lr o (A (R + Z'_{k-1})),  U = R + Z'_M
        (keys are l2-normalized and incoherent; M=14 leaves ~1e-4 worst-case
         truncation, below the fp16/fp32r arithmetic floor)
    O^T = P^T Q^T + U^T G   (one PSUM accumulation group)
    P  += K_rows^T U
    y_chunk = O @ Wo_cols   (fp32r out-projection)

dtypes: projections/out-proj fp32r (tf32-class, 1 cyc/row at N>=256);
state matmuls fp32; Neumann matmuls fp16.
"""

import numpy as np

B, T, D, H = 4, 2048, 1024, 8
d = D // H
HPC = 4            # heads per core
DH = HPC * d       # 512: packed head width
C = 128            # scan chunk
W = 256            # projection window (t)
NWIN = T // W      # 8
NSUB = W // C      # 2 chunks per window
NJ = D // 128      # 8 contraction tiles
NEUMANN_ITERS = 10
EPS = 1e-12

_prog_cache = {}


def _build_program(debug=False):
    def ssl_h(h):
        return slice(h * 128, (h + 1) * 128)

    import concourse.mybir as mybir
    import concourse.tile as tile
    from concourse import bacc
    from concourse.masks import make_identity, make_upper_triangular

    f32 = mybir.dt.float32
    f32r = mybir.dt.float32r
    f16 = mybir.dt.float16
    Alu = mybir.AluOpType
    Act = mybir.ActivationFunctionType

    nc = bacc.Bacc("TRN2", target_bir_lowering=False, debug=False, num_devices=8)

    xT = nc.dram_tensor("xT", [D, T], f16, kind="ExternalInput").ap()
    WqT = nc.dram_tensor("WqT", [D, DH], f16, kind="ExternalInput").ap()
    WkT = nc.dram_tensor("WkT", [D, DH], f16, kind="ExternalInput").ap()
    WvT = nc.dram_tensor("WvT", [D, DH], f16, kind="ExternalInput").ap()
    WlT = nc.dram_tensor("WlT", [D, DH], f16, kind="ExternalInput").ap()
    blr = nc.dram_tensor("blr", [1, DH], f32, kind="ExternalInput").ap()
    WoT = nc.dram_tensor("WoT", [DH, D], f16, kind="ExternalInput").ap()
    y = nc.dram_tensor("y", [T, D], f16, kind="ExternalOutput").ap()
    dbg = {}
    if debug:
        for nm in ("kr0", "vr0", "lr0", "A0", "G0", "R0", "U0", "Ot0", "P0", "kt0", "qt0"):
            dbg[nm] = nc.dram_tensor("dbg_" + nm, [128, DH], f32, kind="ExternalOutput").ap()

    with tile.TileContext(nc) as tc:
        with (
            tc.tile_pool(name="consts", bufs=1) as consts,
            tc.tile_pool(name="weights", bufs=1) as wpool,
            tc.tile_pool(name="state", bufs=1) as state,
            tc.tile_pool(name="xwin", bufs=2) as xwin,
            tc.tile_pool(name="rows", bufs=2) as rows,
            tc.tile_pool(name="twin", bufs=2) as twin,
            tc.tile_pool(name="chunk", bufs=2) as chk,
            tc.tile_pool(name="nscratch", bufs=2) as nsc,
            tc.tile_pool(name="ps_work", bufs=4, space="PSUM") as ps_work,
            tc.tile_pool(name="ps_neu", bufs=2, space="PSUM") as ps_neu,
            tc.tile_pool(name="ps_proj", bufs=2, space="PSUM") as ps_proj,
        ):
            # ---- constants ----
            ident = consts.tile([128, 128], f32, tag="ident")
            make_identity(nc, ident)
            ident16 = consts.tile([128, 128], f16, tag="ident16")
            nc.gpsimd.tensor_copy(ident16[:], ident[:])
            ident32r = consts.tile([128, 128], f32r, tag="ident32r")
            nc.gpsimd.tensor_copy(ident32r[:], ident[:])
            maskA1 = consts.tile([128, 128], f32, tag="maskA1")  # 1 where s<t
            make_upper_triangular(nc, maskA1, val=1.0, diag=False)
            maskG1 = consts.tile([128, 128], f32, tag="maskG1")  # 1 where s<=t
            make_upper_triangular(nc, maskG1, val=1.0, diag=True)
            maskA = consts.tile([128, DH], f32, tag="maskA")
            maskG = consts.tile([128, DH], f32, tag="maskG")
            for h in range(HPC):
                nc.gpsimd.tensor_copy(maskA[:, h * 128:(h + 1) * 128], maskA1[:, :])
                nc.gpsimd.tensor_copy(maskG[:, h * 128:(h + 1) * 128], maskG1[:, :])
            ones_row = consts.tile([1, 128], f16, tag="ones_row")
            nc.vector.memset(ones_row[:], 1.0)
            blr_f32 = consts.tile([1, DH], f32, tag="blr_f32")
            nc.sync.dma_start(blr_f32[:], blr[:])
            blr_sb = consts.tile([1, DH], f16, tag="blr_sb")
            nc.gpsimd.tensor_copy(blr_sb[:], blr_f32[:])

            # ---- resident weights ----
            # wk first (K proj runs first); the rest are issued after window
            # 0's xt DMAs (see late_loads below) so x isn't stuck behind them
            # on the contended DMA engines.
            wq = [wpool.tile([128, DH], f16, tag=f"wq{j}", name=f"wq{j}") for j in range(NJ)]
            wk = [wpool.tile([128, DH], f16, tag=f"wk{j}", name=f"wk{j}") for j in range(NJ)]
            wv = [wpool.tile([128, DH], f16, tag=f"wv{j}", name=f"wv{j}") for j in range(NJ)]
            wl = [wpool.tile([128, DH], f16, tag=f"wl{j}", name=f"wl{j}") for j in range(NJ)]
            wo = [wpool.tile([128, D], f16, tag=f"wo{h}", name=f"wo{h}") for h in range(HPC)]
            for j in range(NJ):
                eng_w = nc.scalar if j % 2 == 0 else nc.sync
                eng_w.dma_start(wk[j][:], WkT[j * 128:(j + 1) * 128, :])

            def late_loads():
                for j in range(NJ):
                    eng_w = nc.scalar if j % 2 == 0 else nc.sync
                    eng_w.dma_start(wq[j][:], WqT[j * 128:(j + 1) * 128, :])
                for j in range(NJ):
                    eng_w = nc.scalar if j % 2 == 0 else nc.sync
                    eng_w.dma_start(wv[j][:], WvT[j * 128:(j + 1) * 128, :])
                for j in range(NJ):
                    eng_w = nc.scalar if j % 2 == 0 else nc.sync
                    eng_w.dma_start(wl[j][:], WlT[j * 128:(j + 1) * 128, :])
                for h in range(HPC):
                    eng_w = nc.scalar if h % 2 == 0 else nc.sync
                    eng_w.dma_start(wo[h][:], WoT[h * 128:(h + 1) * 128, :])

            # ---- state ----
            # P = S^T per head; two independent head-group streams (2 heads each)
            P2 = [state.tile([128, 256], f32, tag=f"P2_{u}", name=f"P2_{u}") for u in range(2)]
            P2h = [state.tile([128, 256], f16, tag=f"P2h_{u}", name=f"P2h_{u}") for u in range(2)]
            for u in range(2):
                nc.vector.memset(P2[u][:], 0.0)
                nc.vector.memset(P2h[u][:], 0.0)

            def norm_part1(ps, raw, ss, col):
                """drain psum, square, reduce sumsq per head into ss[:, col:col+HPC]."""
                nc.scalar.copy(raw[:], ps[:])
                sq = nsc.tile([128, DH], f32, tag="nsq")
                nc.gpsimd.tensor_tensor(sq[:], raw[:], raw[:], Alu.mult)
                nc.vector.tensor_reduce(
                    ss[:, col:col + HPC], sq[:].rearrange("p (h i) -> p h i", h=HPC),
                    axis=mybir.AxisListType.X, op=Alu.add)

            def norm_part2(ss, rcp, n):
                """rcp[:, :n] = 1 / max(sqrt(ss[:, :n]), eps)."""
                nc.scalar.activation(rcp[:, :n], ss[:, :n], Act.Sqrt)
                nc.vector.tensor_scalar(
                    out=rcp[:, :n], in0=rcp[:, :n], scalar1=EPS, scalar2=None,
                    op0=Alu.max)
                nc.vector.reciprocal(rcp[:, :n], rcp[:, :n])

            def norm_scale(raw, rcp, col, out_rows):
                for h in range(HPC):
                    hsl = slice(h * 128, (h + 1) * 128)
                    nc.gpsimd.tensor_tensor(
                        out_rows[:, hsl], raw[:, hsl],
                        rcp[:, col + h:col + h + 1].to_broadcast((128, 128)),
                        Alu.mult)

            def emit_proj(w):
                # ---- load x^T window ----
                xt = [xwin.tile([128, W], f16, tag=f"xt{j}", name=f"xt{j}_{w}") for j in range(NJ)]
                for j in range(NJ):
                    eng_x = nc.sync if j % 2 == 0 else nc.scalar
                    eng_x.dma_start(
                        xt[j][:], xT[j * 128:(j + 1) * 128, w * W:(w + 1) * W])
                if w == 0:
                    late_loads()

                kr =[rows.tile([128, DH], f16, tag=f"kr{s}", name=f"kr{s}_{w}") for s in range(NSUB)]
                vr = [rows.tile([128, DH], f16, tag=f"vr{s}", name=f"vr{s}_{w}") for s in range(NSUB)]
                lr = [rows.tile([128, DH], f32, tag=f"lr{s}", name=f"lr{s}_{w}") for s in range(NSUB)]
                ln = [rows.tile([128, DH], f32, tag=f"ln{s}", name=f"ln{s}_{w}") for s in range(NSUB)]
                ktw = twin.tile([128, HPC * W], f16, tag="ktw")   # [j, (h, t_w)]
                qtw = twin.tile([128, HPC * W], f16, tag="qtw")
                kt3 = ktw[:].rearrange("p (h t) -> p h t", h=HPC)
                qt3 = qtw[:].rearrange("p (h t) -> p h t", h=HPC)

                def proj(wts, tsl, extra_bias=False):
                    ps = ps_proj.tile([128, DH], f32, tag="proj")
                    for j in range(NJ):
                        nc.tensor.matmul(
                            ps[:], xt[j][:, tsl], wts[j][:],
                            start=(j == 0),
                            stop=(j == NJ - 1 and not extra_bias))
                    if extra_bias:
                        nc.tensor.matmul(
                            ps[:], ones_row[:], blr_sb[:], start=False, stop=True)
                    return ps

                # phases: K (sqrt) -> Q+transposes (copies) -> V (sqrt) -> LR (sigmoid)
                # ACT table swaps stay at ~2/window; copies live in every set
                ssK = nsc.tile([128, 2 * HPC], f32, tag="ssK")
                rcpK = nsc.tile([128, 2 * HPC], f32, tag="rcpK")
                rawK = [nsc.tile([128, DH], f32, tag=f"rawK{s}", name=f"rawK{s}_{w}") for s in range(NSUB)]
                for s in range(NSUB):
                    norm_part1(proj(wk, slice(s * 128, (s + 1) * 128)), rawK[s], ssK, s * HPC)
                norm_part2(ssK, rcpK, NSUB * HPC)
                for s in range(NSUB):
                    norm_scale(rawK[s], rcpK, s * HPC, kr[s])

                for s in range(NSUB):
                    psq = proj(wq, slice(s * 128, (s + 1) * 128))
                    qr = nsc.tile([128, DH], f16, tag="qr")
                    nc.scalar.copy(qr[:], psq[:])
                    pst = ps_work.tile([128, DH], f16, tag="work")
                    for h in range(HPC):
                        hsl = slice(h * 128, (h + 1) * 128)
                        nc.tensor.transpose(pst[:, hsl], kr[s][:, hsl], ident16[:])
                    nc.scalar.copy(
                        kt3[:, :, s * 128:(s + 1) * 128],
                        pst[:].rearrange("p (h t) -> p h t", h=HPC))
                    pst2 = ps_work.tile([128, DH], f16, tag="work")
                    for h in range(HPC):
                        hsl = slice(h * 128, (h + 1) * 128)
                        nc.tensor.transpose(pst2[:, hsl], qr[:, hsl], ident16[:])
                    nc.scalar.copy(
                        qt3[:, :, s * 128:(s + 1) * 128],
                        pst2[:].rearrange("p (h t) -> p h t", h=HPC))

                ssV = nsc.tile([128, 2 * HPC], f32, tag="ssV")
                rcpV = nsc.tile([128, 2 * HPC], f32, tag="rcpV")
                rawV = [nsc.tile([128, DH], f32, tag=f"rawV{s}", name=f"rawV{s}_{w}") for s in range(NSUB)]
                for s in range(NSUB):
                    norm_part1(proj(wv, slice(s * 128, (s + 1) * 128)), rawV[s], ssV, s * HPC)
                norm_part2(ssV, rcpV, NSUB * HPC)
                nc.vector.tensor_scalar(
                    out=rcpV[:, :NSUB * HPC], in0=rcpV[:, :NSUB * HPC],
                    scalar1=-1.0, scalar2=None, op0=Alu.mult)
                for s in range(NSUB):
                    norm_scale(rawV[s], rcpV, s * HPC, vr[s])

                for s in range(NSUB):
                    psl = proj(wl, slice(s * 128, (s + 1) * 128), extra_bias=True)
                    nc.scalar.activation(lr[s][:], psl[:], Act.Sigmoid)
                    nc.gpsimd.tensor_scalar(
                        out=ln[s][:], in0=lr[s][:], scalar1=-1.0, scalar2=None,
                        op0=Alu.mult)

                if debug and w == 0:
                    nc.sync.dma_start(dbg["kr0"][:], kr[0][:])
                    nc.sync.dma_start(dbg["vr0"][:], vr[0][:])
                    nc.sync.dma_start(dbg["lr0"][:], lr[0][:])
                    kt_dump = nsc.tile([128, DH], f32, tag="ktdump")
                    qt_dump = nsc.tile([128, DH], f32, tag="qtdump")
                    for h in range(HPC):
                        nc.vector.tensor_copy(kt_dump[:, h * 128:(h + 1) * 128], kt3[:, h, 0:128])
                        nc.vector.tensor_copy(qt_dump[:, h * 128:(h + 1) * 128], qt3[:, h, 0:128])
                    nc.sync.dma_start(dbg["kt0"][:], kt_dump[:])
                    nc.sync.dma_start(dbg["qt0"][:], qt_dump[:])

                return kr, vr, lr, ln, kt3, qt3

            def emit_scan(w, tiles):
                kr, vr, lr, ln, kt3, qt3 = tiles
                # ---- scan chunks (two interleaved head-group streams) ----
                for s in range(NSUB):
                    csl = slice(s * 128, (s + 1) * 128)
                    STR = (slice(0, 256), slice(256, 512))
                    HH = ((0, 1), (2, 3))

                    A2, G2, R2, Rb2, zb2, U2, Ot2 = [], [], [], [], [], [], []
                    for u in range(2):
                        ssl = STR[u]
                        # A = K K^T strict-lower -> fp16
                        psA = ps_work.tile([128, 256], f32, tag="work", name=f"psA{u}_{w}_{s}")
                        for j, h in enumerate(HH[u]):
                            hsl = slice(j * 128, (j + 1) * 128)
                            nc.tensor.matmul(
                                psA[:, hsl], kt3[:, h, csl], kt3[:, h, csl],
                                start=True, stop=True)
                        A4 = chk.tile([128, 256], f16, tag=f"A4_{u}", name=f"A4_{u}_{w}_{s}")
                        nc.vector.tensor_tensor(A4[:], psA[:], maskA[:, ssl], Alu.mult)
                        A2.append(A4)

                        # G = K Q^T masked s<=t (f32)
                        psG = ps_work.tile([128, 256], f32, tag="work", name=f"psG{u}_{w}_{s}")
                        for j, h in enumerate(HH[u]):
                            hsl = slice(j * 128, (j + 1) * 128)
                            nc.tensor.matmul(
                                psG[:, hsl], kt3[:, h, csl], qt3[:, h, csl],
                                start=True, stop=True)
                        G4 = chk.tile([128, 256], f16, tag=f"G4_{u}", name=f"G4_{u}_{w}_{s}")
                        nc.vector.tensor_tensor(G4[:], psG[:], maskG[:, ssl], Alu.mult)
                        G2.append(G4)

                        # Vold = K @ P (rows), R = lr*(V - Vold)
                        psVo = ps_work.tile([128, 256], f32, tag="work", name=f"psVo{u}_{w}_{s}")
                        nc.tensor.matmul(
                            psVo[:], ident16[:], vr[s][:, ssl],
                            start=True, stop=False)
                        for j, h in enumerate(HH[u]):
                            hsl = slice(j * 128, (j + 1) * 128)
                            nc.tensor.matmul(
                                psVo[:, hsl], kt3[:, h, csl], P2h[u][:, hsl],
                                start=False, stop=True)
                        R4 = chk.tile([128, 256], f32, tag=f"R4_{u}", name=f"R4_{u}_{w}_{s}")
                        nc.vector.tensor_tensor(R4[:], ln[s][:, ssl], psVo[:], Alu.mult)
                        Rb = chk.tile([128, 256], f16, tag=f"Rb_{u}", name=f"Rb_{u}_{w}_{s}")
                        nc.gpsimd.tensor_copy(Rb[:], R4[:])
                        R2.append(R4)
                        Rb2.append(Rb)
                        zb2.append(None)

                    # Neumann/Horner, streams interleaved per iteration:
                    # Z'_k = -lr o (A @ (R + Z'_{k-1}))
                    for it in range(NEUMANN_ITERS):
                        psN2 = []
                        for u in range(2):
                            psN = ps_neu.tile([128, 256], f32, tag="neu", name=f"psN{u}_{w}_{s}_{it}")
                            for j in range(2):
                                hsl = slice(j * 128, (j + 1) * 128)
                                nc.tensor.matmul(
                                    psN[:, hsl], A2[u][:, hsl], Rb2[u][:, hsl],
                                    start=True, stop=(zb2[u] is None))
                                if zb2[u] is not None:
                                    nc.tensor.matmul(
                                        psN[:, hsl], A2[u][:, hsl], zb2[u][:, hsl],
                                        start=False, stop=True)
                            psN2.append(psN)
                        for u in range(2):
                            zb_new = chk.tile([128, 256], f16, tag=f"zb_{u}", name=f"zb_{u}_{w}_{s}_{it}")
                            nc.vector.tensor_tensor(zb_new[:], ln[s][:, STR[u]], psN2[u][:], Alu.mult)
                            zb2[u] = zb_new

                    for u in range(2):
                        U4 = chk.tile([128, 256], f16, tag=f"U4_{u}", name=f"U4_{u}_{w}_{s}")
                        nc.gpsimd.tensor_tensor(U4[:], R2[u][:], zb2[u][:], Alu.add)
                        U2.append(U4)

                        # O^T = P^T Q^T + U^T G   [i, (h,t)]
                        psO = ps_work.tile([128, 256], f32, tag="work", name=f"psO{u}_{w}_{s}")
                        for j, h in enumerate(HH[u]):
                            hsl = slice(j * 128, (j + 1) * 128)
                            nc.tensor.matmul(
                                psO[:, hsl], P2h[u][:, hsl], qt3[:, h, csl],
                                start=True, stop=False)
                            nc.tensor.matmul(
                                psO[:, hsl], U4[:, hsl], G2[u][:, hsl],
                                start=False, stop=True)
                        Ot = chk.tile([128, 256], f16, tag=f"Ot_{u}", name=f"Ot_{u}_{w}_{s}")
                        nc.scalar.copy(Ot[:], psO[:])
                        Ot2.append(Ot)

                        # P += K_rows^T U
                        psP = ps_work.tile([128, 256], f32, tag="work", name=f"psP{u}_{w}_{s}")
                        for j, h in enumerate(HH[u]):
                            hsl = slice(j * 128, (j + 1) * 128)
                            nc.tensor.matmul(
                                psP[:, hsl], kr[s][:, ssl_h(h)], U4[:, hsl],
                                start=True, stop=True)
                        nc.vector.tensor_tensor(P2[u][:], P2[u][:], psP[:], Alu.add)
                        nc.scalar.copy(P2h[u][:], P2[u][:])

                    # y_chunk = O @ Wo_cols   [t, o]
                    t0 = w * W + s * 128
                    for ot in range(2):
                        osl = slice(ot * 512, (ot + 1) * 512)
                        psy = ps_work.tile([128, 512], f32, tag="work", name=f"psy{ot}_{w}_{s}")
                        for h in range(HPC):
                            u, j = divmod(h, 2)
                            hsl = slice(j * 128, (j + 1) * 128)
                            nc.tensor.matmul(
                                psy[:], Ot2[u][:, hsl], wo[h][:, osl],
                                start=(h == 0), stop=(h == HPC - 1))
                        y_sb = chk.tile([128, 512], f16, tag=f"y_sb{ot}", name=f"ysb{ot}_{w}_{s}")
                        nc.scalar.copy(y_sb[:], psy[:])
                        nc.sync.dma_start(y[t0:t0 + 128, osl], y_sb[:])


            for w in range(NWIN):
                emit_scan(w, emit_proj(w))

    nc.compile()
    return nc


def get_program(debug=False):
    key = "nc_dbg" if debug else "nc"
    if key not in _prog_cache:
        _prog_cache[key] = _build_program(debug)
    return _prog_cache[key]


def kernel(x, Wq, Wk, Wv, Wo, Wlr, b_lr):
    from concourse import bass_utils

    nc = get_program()
    x = np.asarray(x, np.float16)
    Wq = np.asarray(Wq, np.float16)
    Wk = np.asarray(Wk, np.float16)
    Wv = np.asarray(Wv, np.float16)
    Wo = np.asarray(Wo, np.float16)
    Wlr = np.asarray(Wlr, np.float16)
    b_lr = np.asarray(b_lr, np.float32)

    in_maps = []
    for c in range(8):
        b, hg = divmod(c, 2)
        rs = slice(hg * DH, (hg + 1) * DH)   # head-sliced output rows of W*
        in_maps.append({
            "xT": np.ascontiguousarray(x[b].T),
            "WqT": np.ascontiguousarray(Wq[rs, :].T),
            "WkT": np.ascontiguousarray(Wk[rs, :].T),
            "WvT": np.ascontiguousarray(Wv[rs, :].T),
            "WlT": np.ascontiguousarray(Wlr[rs, :].T),
            "blr": np.ascontiguousarray(b_lr[rs][None, :]),
            "WoT": np.ascontiguousarray(Wo[:, rs].T),
        })
    res = bass_utils.run_bass_kernel_spmd(nc, in_maps, core_ids=list(range(8)))
    out = np.empty((B, T, D), np.float32)
    for b in range(B):
        out[b] = (res.results[2 * b]["y"].astype(np.float32)
                  + res.results[2 * b + 1]["y"].astype(np.float32))
    return out

